# revision 1
# baseline (speedup 1.0000x reference)
"""MoE top-1 routing kernel for Trainium2 (8 NeuronCores, expert-parallel).

Model (E=8, D=512, F=2048, N=4096):
    logits = x @ Wg + bg; e = argmax(logits)
    y[i] = relu(x[i] @ W1[e] + b1[e]) @ W2[e] + b2[e]

Strategy:
- Host computes the gate (f64 matmul + argmax) and routes tokens; core e gets
  only expert e's tokens (padded to the max expert count C) + expert e's
  weights, and runs a dense 2-layer MLP in fp32r (full-rate fp32 matmul mode,
  ~2e-4 scale-relative error).
- All tensors are host-packed into SBUF-native [128, *] layouts so every DMA
  moves multi-KB contiguous runs per partition (one dma_start per piece).
- DMA pieces are issued in consumption order (x chunk 0, W1 by f-groups,
  W2 by fo-groups) so matmuls start ~5us in; stage-2 accumulation is emitted
  in W2-piece arrival order, interleaved across PSUM banks.
- A short dummy-matmul burst warms the PE clock (HAM) during the DMA head.
- Tokens ride the matmul free dim in chunks of <=512 columns (PSUM bank
  limit), >=256 wide where possible (fp32r full-rate threshold).
"""

import sys

sys.path.insert(0, "/opt/trn_rl_repo")

import numpy as np

E, D, F, N_CORES = 8, 512, 2048, 8
KD, KF = D // 128, F // 128  # 4, 16
G1, G2 = KF // 4, KF // 4    # w1 f-piece count, w2 fo-piece count (4 each)

_cache: dict = {}


def _build(C: int, chunks: list[tuple[int, int]]):
    import concourse.tile as tile
    import concourse.mybir as mybir
    from concourse import bacc

    f32, f32r = mybir.dt.float32, mybir.dt.float32r
    Relu = mybir.ActivationFunctionType.Relu

    nc = bacc.Bacc("TRN2", target_bir_lowering=False, debug=False)
    # packed layouts, all [128, *]:
    #   xTi[p, chunk_off + ko*cw + c] = x_e[c0+c, 128*ko+p]
    #   w1i[p, g*2048 + ko*512 + fi]  = W1_e[128*ko+p, 512*g+fi]
    #   w2i[p, h*2048 + j*512 + d]    = W2_e[128*(4h+j)+p, d]
    #   bi[p, f] = b1_e[128f+p] (f<16);  bi[p, 16+d] = b2_e[128d+p]
    #   yTi[p, d*C + c] = y_e[c, 128d+p]
    xTi = nc.dram_tensor("xTi", [128, KD * C], f32r, kind="ExternalInput").ap()
    w1i = nc.dram_tensor("w1i", [128, KD * F], f32r, kind="ExternalInput").ap()
    w2i = nc.dram_tensor("w2i", [128, KF * D], f32r, kind="ExternalInput").ap()
    bi = nc.dram_tensor("bi", [128, KF + KD], f32, kind="ExternalInput").ap()
    yTi = nc.dram_tensor("yTi", [128, KD * C], f32, kind="ExternalOutput").ap()
    y3 = yTi.rearrange("p (d c) -> p d c", c=C)

    with tile.TileContext(nc) as tc:
        with tc.tile_pool(name="wp", bufs=1) as wp, \
             tc.tile_pool(name="hp", bufs=1) as hp, \
             tc.tile_pool(name="yp", bufs=2) as yp, \
             tc.tile_pool(name="scr", bufs=1) as scr, \
             tc.tile_pool(name="pp", bufs=3, space="PSUM") as pp:

            # --- PE warm-up: dummy matmuls during the DMA head (HAM ramp).
            # f32 runs 4 cycles/row: N=128 -> ~427ns cold each, so 9 of them
            # cover the ~3.4us HAM window while delaying real matmuls <0.5us.
            wrm = scr.tile([128, 128], f32, name="wrm")
            nc.vector.memset(wrm[:], 0.0)
            wps = pp.tile([128, 128], f32, name="wps", tag="wps", bufs=1)
            for _ in range(14):
                nc.tensor.matmul(wps[:], wrm[:], wrm[:], start=True, stop=True)

            # --- DMA issue, consumption order, single engine (sync) ---
            bis = wp.tile([128, KF + KD], f32, name="bis")
            nc.scalar.dma_start(bis[:], bi[:])

            w1t = wp.tile([128, KD * F], f32r, name="w1t")
            w2t = wp.tile([128, KF * D], f32r, name="w2t")
            xs = []
            off = 0
            for ci, (c0, c1) in enumerate(chunks):
                cw = c1 - c0
                xst = wp.tile([128, KD * cw], f32r, name=f"xs{ci}", tag=f"xs{ci}")
                xs.append((xst, off))
                off += KD * cw
            # Issue order = consumption order, serial on sync (parallel
            # multi-engine issue measured slower; aggregate is HBM-bound at
            # ~390GB/s once a few dma_starts are outstanding).
            def dma_piece(dst, src, lo, hi):
                nc.sync.dma_start(dst[:, lo:hi], src[:, lo:hi])

            xst0, o0 = xs[0]
            half = xst0.shape[-1] // 2
            nc.sync.dma_start(xst0[:, :half], xTi[:, o0:o0 + half])
            dma_piece(w1t, w1i, 0, 1024)
            nc.sync.dma_start(xst0[:, half:], xTi[:, o0 + half:o0 + xst0.shape[-1]])
            dma_piece(w1t, w1i, 1024, 2048)
            if len(xs) > 1:
                xst1, o1 = xs[1]
                nc.sync.dma_start(xst1[:], xTi[:, o1:o1 + xst1.shape[-1]])
            for g in range(1, G1):
                dma_piece(w1t, w1i, g * 2048, g * 2048 + 1024)
                dma_piece(w1t, w1i, g * 2048 + 1024, (g + 1) * 2048)
            for xst, o in xs[2:]:
                nc.sync.dma_start(xst[:], xTi[:, o:o + xst.shape[-1]])
            for h in range(G2):
                dma_piece(w2t, w2i, h * 2048, h * 2048 + 1024)
                dma_piece(w2t, w2i, h * 2048 + 1024, (h + 1) * 2048)

            # --- stage 1: h = relu(x @ W1 + b1), emitted in w1-piece order ---
            hs = {}  # (ci, f) -> tile
            for g in range(G1):
                for ci, (c0, c1) in enumerate(chunks):
                    cw = c1 - c0
                    xst = xs[ci][0]
                    for f in range(4 * g, 4 * g + 4):
                        p1 = pp.tile([128, cw], f32, name=f"p1_{ci}_{f}", tag="p1")
                        for ko in range(KD):
                            lhsT = w1t[:, g * 2048 + ko * 512 + (f % 4) * 128:
                                       g * 2048 + ko * 512 + (f % 4) * 128 + 128]
                            nc.tensor.matmul(p1[:], lhsT, xst[:, ko * cw:(ko + 1) * cw],
                                             start=(ko == 0), stop=(ko == KD - 1))
                        h = hp.tile([128, cw], f32r, name=f"h{ci}_{f}",
                                    tag=f"h{ci % 2}_{f}")
                        nc.scalar.activation(h[:], p1[:], Relu, bias=bis[:, f:f + 1])
                        hs[(ci, f)] = h

            # --- stage 2: y = h @ W2 + b2 ---
            # Early chunks run w2-piece-arrival-major (h outer) to follow the
            # DMA stream; the last chunk runs d-major (w2 fully resident by
            # then) so each d's bias-add + output DMA starts as soon as that
            # d finishes, overlapping the kernel tail.
            for ci, (c0, c1) in enumerate(chunks):
                cw = c1 - c0
                last = ci == len(chunks) - 1
                p2s = [pp.tile([128, cw], f32, name=f"p2_{ci}_{d}", tag=f"p2_{d}",
                               bufs=1) for d in range(KD)]
                ys = yp.tile([128, KD, cw], f32, name=f"ys{ci}", tag="ys")
                out_engs = [nc.gpsimd, nc.scalar, nc.gpsimd, nc.scalar]

                def s2_mm(d, fo):
                    h2, j = divmod(fo, 4)
                    lhsT = w2t[:, h2 * 2048 + j * 512 + d * 128:
                               h2 * 2048 + j * 512 + d * 128 + 128]
                    nc.tensor.matmul(p2s[d][:], lhsT, hs[(ci, fo)][:],
                                     start=(fo == 0), stop=(fo == KF - 1))

                def s2_out(d):
                    nc.vector.tensor_scalar_add(ys[:, d, :], p2s[d][:],
                                                bis[:, KF + d:KF + d + 1])
                    out_engs[d].dma_start(y3[:, d, c0:c1], ys[:, d, :])

                if last:
                    for d in range(KD):
                        for fo in range(KF):
                            s2_mm(d, fo)
                        s2_out(d)
                else:
                    for h2 in range(G2):
                        for d in range(KD):
                            for j in range(4):
                                s2_mm(d, 4 * h2 + j)
                    for d in range(KD):
                        s2_out(d)
    nc.compile()
    return nc


def _plan_chunks(C: int) -> list[tuple[int, int]]:
    n = max(1, -(-C // 512))
    base, rem = divmod(C, n)
    out, pos = [], 0
    for i in range(n):
        w = base + (1 if i < rem else 0)
        out.append((pos, pos + w))
        pos += w
    return out


def _get_nc(C: int):
    if C not in _cache:
        _cache[C] = _build(C, _plan_chunks(C))
    return _cache[C]


def _pack_inputs(x, W1, b1, W2, b2, idx, order, starts, C):
    chunks = _plan_chunks(C)
    in_maps, toks_per_core = [], []
    for e in range(E):
        toks = order[starts[e]:starts[e + 1]]
        toks_per_core.append(toks)
        xe = np.zeros((C, D), np.float32)
        xe[:len(toks)] = x[toks]
        xeT = xe.T  # [D, C]
        xTi = np.concatenate(
            [xeT[:, c0:c1].reshape(KD, 128, c1 - c0).transpose(1, 0, 2)
             .reshape(128, KD * (c1 - c0)) for c0, c1 in chunks], axis=1)
        w1p = np.concatenate(
            [W1[e][:, 512 * g:512 * (g + 1)].reshape(KD, 128, 512)
             .transpose(1, 0, 2).reshape(128, KD * 512) for g in range(G1)], axis=1)
        w2p = np.concatenate(
            [W2[e][512 * h:512 * (h + 1), :].reshape(4, 128, 512)
             .transpose(1, 0, 2).reshape(128, 4 * 512) for h in range(G2)], axis=1)
        bi = np.concatenate([b1[e].reshape(KF, 128).T,
                             b2[e].reshape(KD, 128).T], axis=1)
        in_maps.append({
            "xTi": np.ascontiguousarray(xTi),
            "w1i": np.ascontiguousarray(w1p),
            "w2i": np.ascontiguousarray(w2p),
            "bi": np.ascontiguousarray(bi),
        })
    return in_maps, toks_per_core, chunks


def kernel(x, Wg, bg, W1, b1, W2, b2):
    from concourse.bass_utils import run_bass_kernel_spmd

    x = np.asarray(x, dtype=np.float32)
    n_tok = x.shape[0]

    # host gate in f64: the mathematically-true argmax
    logits = x.astype(np.float64) @ np.asarray(Wg, np.float64) + np.asarray(bg, np.float64)
    idx = logits.argmax(1)

    counts = np.bincount(idx, minlength=E)
    order = np.argsort(idx, kind="stable")
    starts = np.zeros(E + 1, np.int64)
    starts[1:] = np.cumsum(counts)

    C = max(int(counts.max()), 256)
    C = (C + 15) // 16 * 16

    W1 = np.asarray(W1, np.float32)
    W2 = np.asarray(W2, np.float32)
    b1 = np.asarray(b1, np.float32)
    b2 = np.asarray(b2, np.float32)

    in_maps, toks_per_core, chunks = _pack_inputs(x, W1, b1, W2, b2,
                                                  idx, order, starts, C)
    nc = _get_nc(C)
    res = run_bass_kernel_spmd(nc, in_maps, core_ids=list(range(N_CORES)))

    out = np.zeros((n_tok, D), np.float32)
    for e in range(E):
        toks = toks_per_core[e]
        ye = res.results[e]["yTi"].reshape(128, KD, C).transpose(2, 1, 0) \
            .reshape(C, D)
        out[toks] = ye[:len(toks)]
    return out



# revision 3
# speedup vs baseline: 1.1268x; 1.1268x over previous
"""MoE top-1 routing kernel for Trainium2 (8 NeuronCores, expert-F-sharded).

Model (E=8, D=512, F=2048, N=4096):
    logits = x @ Wg + bg; e = argmax(logits)
    y[i] = relu(x[i] @ W1[e] + b1[e]) @ W2[e] + b2[e]

Strategy (v2 — bf16 + quarter-F expert sharding):
- Host computes the gate (f64 matmul + argmax) and routes tokens.
- Each expert's FFN is split into 4 F-quarters (Fs=512). Experts are paired
  hot-with-cold into 4 "slots"; slot s appears on every core with the same
  compile-time token width W_s = max count over the slot's two experts.
  Core j, slot s holds (expert = pair[s][j//4], quarter q = j%4) and
  processes ALL of that expert's tokens against its F-quarter.
  PE work per core = sum_s W_s * Fs / 16 cycles — near count-independent,
  so the 622-vs-426 expert imbalance no longer pads every core.
- Everything on the wire is bf16 (halves HBM traffic vs fp32 and enables
  FWL fast weight load; rel-err ~2e-3 vs the 2e-2 gate). PSUM accumulates
  fp32; b1 is applied in the Relu, b2 is added only by the q==0 cell
  (zeros elsewhere), partial y's are summed on the host in fp32.
- Software-pipelined emission: stage1 of chunk i+1 is emitted between
  stage1 and stage2 of chunk i so the PE never waits on the Relu latency.
- A dummy-matmul burst warms the PE clock (HAM) during the DMA head.
"""

import sys

sys.path.insert(0, "/opt/trn_rl_repo")

import numpy as np
import ml_dtypes

BF16 = ml_dtypes.bfloat16
E, D, F, N_CORES = 8, 512, 2048, 8
KD = D // 128      # 4 contraction tiles (stage1) == output d tiles (stage2)
FS = 512           # F-columns per slot (quarter of F)
KQ = FS // 128     # 4
NSLOT = 4

_cache: dict = {}


def _chunks_of(w: int) -> list[int]:
    # split width into <=512-col chunks (PSUM bank limit), evenly
    if w <= 512:
        return [w]
    n = -(-w // 512)
    base = (-(-w // n) + 15) // 16 * 16
    out, rem = [], w
    while rem > 0:
        c = min(base, rem)
        out.append(c)
        rem -= c
    return out


def _build(widths: tuple[int, ...]):
    import concourse.tile as tile
    import concourse.mybir as mybir
    from concourse import bacc

    f32 = mybir.dt.float32
    bf16 = mybir.dt.bfloat16
    Relu = mybir.ActivationFunctionType.Relu

    nc = bacc.Bacc("TRN2", target_bir_lowering=False, debug=False)

    # Per-slot dram tensors (bf16 except biases):
    #   w{s}[p, f*512 + ko*128 + c]        = W1[e][128*ko+p, 512*q + 128*f + c]
    #   w{s}[p, 2048 + d*512 + fo*128 + c] = W2[e][512*q + 128*fo + p, 128*d + c]
    #   x{s}[p, KD*lo + ko*cw + c]         = x[tok_{lo+c}, 128*ko + p]
    #   b[p, s*8+f] = b1[e][512*q+128*f+p];  b[p, s*8+4+d] = b2[e][128*d+p] (q==0 else 0)
    #   y{s}[p, d*W + c] = partial y[tok_c, 128*d+p]
    w_d = [nc.dram_tensor(f"w{s}", [128, 2 * KD * FS], bf16, kind="ExternalInput").ap()
           for s in range(NSLOT)]
    x_d = [nc.dram_tensor(f"x{s}", [128, KD * widths[s]], bf16, kind="ExternalInput").ap()
           for s in range(NSLOT)]
    b_d = nc.dram_tensor("b", [128, NSLOT * 8], f32, kind="ExternalInput").ap()
    y_d = [nc.dram_tensor(f"y{s}", [128, KD * widths[s]], bf16, kind="ExternalOutput").ap()
           for s in range(NSLOT)]
    y3 = [y_d[s].rearrange("p (d c) -> p d c", c=widths[s]) for s in range(NSLOT)]

    CH = []  # (slot, lo, cw)
    for s, w in enumerate(widths):
        lo = 0
        for cw in _chunks_of(w):
            CH.append((s, lo, cw))
            lo += cw
    n_ch = len(CH)

    with tile.TileContext(nc) as tc:
        with tc.tile_pool(name="wp", bufs=1) as wp, \
             tc.tile_pool(name="hp", bufs=2) as hp, \
             tc.tile_pool(name="yp", bufs=2) as yp, \
             tc.tile_pool(name="scr", bufs=1) as scr, \
             tc.tile_pool(name="pp", bufs=3, space="PSUM") as pp:

            # --- PE warm-up: dummy matmuls during the DMA head (HAM ramp).
            wrm = scr.tile([128, 256], bf16, name="wrm")
            nc.vector.memset(wrm[:], 0.0)
            wps = pp.tile([128, 256], f32, name="wps", tag="wps", bufs=1)
            for _ in range(14):
                nc.tensor.matmul(wps[:], wrm[:, :128], wrm[:], start=True, stop=True)

            # --- DMA issue, consumption order, sync engine (HWDGE) ---
            bis = wp.tile([128, NSLOT * 8], f32, name="bis")
            nc.scalar.dma_start(bis[:], b_d[:])

            wt = [wp.tile([128, 2 * KD * FS], bf16, name=f"wt{s}") for s in range(NSLOT)]
            xt = [wp.tile([128, KD * widths[s]], bf16, name=f"xt{s}") for s in range(NSLOT)]

            for s in range(NSLOT):
                # w1 in 2 pieces (f0-1, f2-3), x per chunk, w2 in 2 pieces (d0-1, d2-3)
                nc.sync.dma_start(wt[s][:, 0:1024], w_d[s][:, 0:1024])
                xlo = 0
                xpieces = []
                for cw in _chunks_of(widths[s]):
                    xpieces.append((KD * xlo, KD * (xlo + cw)))
                    xlo += cw
                nc.sync.dma_start(xt[s][:, xpieces[0][0]:xpieces[0][1]],
                                  x_d[s][:, xpieces[0][0]:xpieces[0][1]])
                nc.sync.dma_start(wt[s][:, 1024:2048], w_d[s][:, 1024:2048])
                for lo_, hi_ in xpieces[1:]:
                    nc.sync.dma_start(xt[s][:, lo_:hi_], x_d[s][:, lo_:hi_])
                nc.sync.dma_start(wt[s][:, 2048:3072], w_d[s][:, 2048:3072])
                nc.sync.dma_start(wt[s][:, 3072:4096], w_d[s][:, 3072:4096])

            # --- compute, software-pipelined: st1(i+1) between st1(i) and st2(i)
            hs = {}

            def st1(ci):
                s, lo, cw = CH[ci]
                for f in range(KQ):
                    p1 = pp.tile([128, 512], f32, name=f"p1_{ci}_{f}", tag="p1")
                    for ko in range(KD):
                        lhsT = wt[s][:, f * 512 + ko * 128: f * 512 + ko * 128 + 128]
                        rhs = xt[s][:, KD * lo + ko * cw: KD * lo + (ko + 1) * cw]
                        nc.tensor.matmul(p1[:, :cw], lhsT, rhs,
                                         start=(ko == 0), stop=(ko == KD - 1))
                    h = hp.tile([128, 512], bf16, name=f"h{ci}_{f}", tag=f"h{f}")
                    nc.scalar.activation(h[:, :cw], p1[:, :cw], Relu,
                                         bias=bis[:, s * 8 + f: s * 8 + f + 1])
                    hs[(ci, f)] = h

            def st2(ci, last):
                s, lo, cw = CH[ci]
                ys = yp.tile([128, KD, 512], bf16, name=f"ys{ci}", tag="ys")
                for d in range(KD):
                    p2 = pp.tile([128, 512], f32, name=f"p2_{ci}_{d}",
                                 tag=f"p2_{d}", bufs=1)
                    for fo in range(KQ):
                        lhsT = wt[s][:, 2048 + d * 512 + fo * 128:
                                     2048 + d * 512 + fo * 128 + 128]
                        nc.tensor.matmul(p2[:, :cw], lhsT, hs[(ci, fo)][:, :cw],
                                         start=(fo == 0), stop=(fo == KQ - 1))
                    nc.vector.tensor_scalar_add(ys[:, d, :cw], p2[:, :cw],
                                                bis[:, s * 8 + 4 + d: s * 8 + 4 + d + 1])
                    if last:
                        eng = [nc.gpsimd, nc.scalar, nc.gpsimd, nc.sync][d]
                        eng.dma_start(y3[s][:, d, lo:lo + cw], ys[:, d, :cw])
                if not last:
                    eng = nc.gpsimd if ci % 2 == 0 else nc.scalar
                    eng.dma_start(y3[s][:, :, lo:lo + cw], ys[:, :, :cw])

            st1(0)
            for i in range(n_ch):
                if i + 1 < n_ch:
                    st1(i + 1)
                st2(i, last=(i == n_ch - 1))

    nc.compile()
    return nc


def _get_nc(widths: tuple[int, ...]):
    if widths not in _cache:
        _cache[widths] = _build(widths)
    return _cache[widths]


def _plan(counts):
    """Pair adjacent experts in sorted order into NSLOT slots (minimizes
    sum of per-slot maxima); return (pairs, widths)."""
    order = np.argsort(-counts, kind="stable")
    pairs = [(int(order[2 * s]), int(order[2 * s + 1])) for s in range(NSLOT)]
    widths = tuple(
        (max(int(counts[a]), int(counts[b]), 16) + 15) // 16 * 16
        for a, b in pairs)
    return pairs, widths


def _pack_inputs(x, W1, b1, W2, b2, order, starts, pairs, widths):
    """Build per-core in_maps. Core j, slot s: expert pair[s][j//4], quarter j%4."""
    xbf = x.astype(BF16)
    in_maps = []
    toks_of = [order[starts[e]:starts[e + 1]] for e in range(E)]
    # per (expert, quarter) packed weights, shared across the 2 cores... each
    # (e, q) appears on exactly one core, so just build per core.
    for j in range(N_CORES):
        q = j % 4
        m = {}
        bcols = np.zeros((128, NSLOT * 8), np.float32)
        for s in range(NSLOT):
            e = pairs[s][0] if j < 4 else pairs[s][1]
            W = widths[s]
            # w1 (f-major): [p, f*512 + ko*128 + c]
            w1s = W1[e][:, FS * q: FS * (q + 1)]               # [512, 512] (D, Fs)
            w1p = w1s.reshape(KD, 128, KQ, 128).transpose(1, 2, 0, 3).reshape(128, KD * FS)
            # w2 (d-major): [p, d*512 + fo*128 + c]
            w2s = W2[e][FS * q: FS * (q + 1), :]               # [512, 512] (Fs, D)
            w2p = w2s.reshape(KQ, 128, KD, 128).transpose(1, 2, 0, 3).reshape(128, KQ * D)
            m[f"w{s}"] = np.ascontiguousarray(
                np.concatenate([w1p, w2p], axis=1).astype(BF16))
            # x: chunk-major, ko within chunk
            toks = toks_of[e]
            xe = np.zeros((W, D), BF16)
            xe[:len(toks)] = xbf[toks]
            xeT = xe.T                                          # [D, W]
            lo, xparts = 0, []
            for cw in _chunks_of(W):
                xparts.append(xeT[:, lo:lo + cw].reshape(KD, 128, cw)
                              .transpose(1, 0, 2).reshape(128, KD * cw))
                lo += cw
            m[f"x{s}"] = np.ascontiguousarray(np.concatenate(xparts, axis=1))
            # biases
            bcols[:, s * 8: s * 8 + KQ] = b1[e][FS * q: FS * (q + 1)].reshape(KQ, 128).T
            if q == 0:
                bcols[:, s * 8 + 4: s * 8 + 8] = b2[e].reshape(KD, 128).T
        m["b"] = bcols
        in_maps.append(m)
    return in_maps, toks_of


def kernel(x, Wg, bg, W1, b1, W2, b2):
    from concourse.bass_utils import run_bass_kernel_spmd

    x = np.asarray(x, dtype=np.float32)
    n_tok = x.shape[0]

    # host gate in f64: the mathematically-true argmax
    logits = x.astype(np.float64) @ np.asarray(Wg, np.float64) + np.asarray(bg, np.float64)
    idx = logits.argmax(1)

    counts = np.bincount(idx, minlength=E)
    order = np.argsort(idx, kind="stable")
    starts = np.zeros(E + 1, np.int64)
    starts[1:] = np.cumsum(counts)

    pairs, widths = _plan(counts)

    W1 = np.asarray(W1, np.float32)
    W2 = np.asarray(W2, np.float32)
    b1 = np.asarray(b1, np.float32)
    b2 = np.asarray(b2, np.float32)

    in_maps, toks_of = _pack_inputs(x, W1, b1, W2, b2, order, starts, pairs, widths)
    nc = _get_nc(widths)
    res = run_bass_kernel_spmd(nc, in_maps, core_ids=list(range(N_CORES)))

    out = np.zeros((n_tok, D), np.float32)
    for j in range(N_CORES):
        for s in range(NSLOT):
            e = pairs[s][0] if j < 4 else pairs[s][1]
            W = widths[s]
            toks = toks_of[e]
            ye = res.results[j][f"y{s}"].astype(np.float32) \
                .reshape(128, KD, W).transpose(2, 1, 0).reshape(W, D)
            out[toks] += ye[:len(toks)]
    return out
